# revision 15
# baseline (speedup 1.0000x reference)
"""Trainium2 Bass kernel for nn_CrossAttention (4-layer MLP -> cross-attention).

Sharding: data-parallel across batch B=8, one batch element per NeuronCore.

Layout strategy (per core):
  - activations flow feature-major (transposed): the MLP chain
    h_{l+1}^T = W_l^T @ h_l^T needs no transposes (W natural [K,M] = lhsT),
  - scores are computed transposed (scores^T = k @ q^T, kv on partitions),
  - v is computed token-major from y8/Wv8 (fp8 DR) and stored kv-pair-packed,
  - the attention output uses the decomposition
        E = exp(s) = 1 + F,   out = (colsum(v) + F @ v) / (2048 + sum F) + bv
    with F' = 16*F = exp(s + ln16) - 16 stored in fp8 (scale 16 keeps F'
    clear of the e4m3 subnormal floor); colsum(v) = (16*ysum^T Wv) in bf16
    from the true y (ysum streamed+reduced from yT), so the fp8 noise of v
    only enters multiplied by |F| ~ 0.02 instead of E ~ 1.  All the alpha
    factors cancel: out = (po + CS16)/(32768 + pS') + bv.

Precision: every big matmul runs fp8(e4m3) DoubleRow (2 contraction rows
per PE pass -> 2x FLOP rate at the same column stream rate); errors on the
MLP/q/k path only perturb attention weights, the F-decomposition suppresses
the v/E quantization error by sigma_F ~ 0.02. All accumulation fp32 in PSUM.

fp8 operands are pair-packed for DoubleRow: logical contraction index
k = (2t+r)*128+p lives in tile t, partition p, middle index r, i.e. SBUF
tiles [128, 2, N] (weights pre-packed on host to [K/2, 2*N]).

Activations are split between the Scalar and Vector engines (alternating
free-dim blocks) so neither stalls the tensor engine.
"""

import sys

if "/opt/trn_rl_repo" not in sys.path:
    sys.path.insert(0, "/opt/trn_rl_repo")

import numpy as np
import ml_dtypes

P = 128
D = 1024
DB = 512
S = 2048
KD = D // P       # 8 feature tiles of 128
KB = DB // P      # 4
PD = KD // 2      # 4 fp8 pair-tiles for a 1024 contraction
PB = KB // 2      # 2 for 512
NT = S // P       # 16 token tiles
PT = NT // 2      # 8 kv pair tiles
NB = 512          # moving-operand free-dim block
NBLK = S // NB    # 4 token blocks
HALF = S // 2     # q processed in 2 halves during attention
NCORES = 8
SCALE = float(1.0 / np.sqrt(D))
ALPHA = 16.0
LN_ALPHA = float(np.log(ALPHA))
DEN_CONST = ALPHA * S   # 32768

BF16 = ml_dtypes.bfloat16
FP8 = ml_dtypes.float8_e4m3

_NC = None


def build_nc():
    """Build + compile the per-core Bass program (cached)."""
    global _NC
    if _NC is not None:
        return _NC

    from contextlib import ExitStack
    import concourse.bass as bass
    import concourse.tile as tile
    from concourse import bacc, mybir

    BF = mybir.dt.bfloat16
    F8 = mybir.dt.float8e4
    F32 = mybir.dt.float32
    AF = mybir.ActivationFunctionType
    ALU = mybir.AluOpType
    DR = mybir.MatmulPerfMode.DoubleRow

    nc = bacc.Bacc("TRN2", target_bir_lowering=False, debug=False,
                   num_devices=NCORES)

    def din(name, shape, dt):
        return nc.dram_tensor(name, shape, dt, kind="ExternalInput").ap()

    # fp8 operands arrive pair-packed: [K/2, 2*N]
    x8d = din("x8", [D // 2, 2 * S], F8)
    y8d = din("y8", [D // 2, 2 * S], F8)
    yTd = din("yT", [D, S], BF)
    W1d = din("W1", [D // 2, 2 * D], F8)
    W2d = din("W2", [D // 2, 2 * DB], F8)
    W3d = din("W3", [DB // 2, 2 * D], F8)
    W4d = din("W4", [D // 2, 2 * D], F8)
    Wqd = din("Wq", [D // 2, 2 * D], F8)
    Wkd = din("Wk", [D // 2, 2 * D], F8)
    Wv8d = din("Wv8", [D // 2, 2 * D], F8)
    Wvbd = din("Wvb", [D, D], BF)
    b1 = din("b1", [P, KD], F32)
    b2 = din("b2", [P, KB], F32)
    b3 = din("b3", [P, KD], F32)
    b4 = din("b4", [P, KD], F32)
    bq = din("bq", [P, KD], F32)
    bk = din("bk", [P, KD], F32)
    bv = din("bv", [D], F32)
    out = nc.dram_tensor("out", [S, D], BF, kind="ExternalOutput").ap()

    with tile.TileContext(nc) as tc, ExitStack() as ctx:
        small = ctx.enter_context(tc.tile_pool(name="small", bufs=1))
        rpool = ctx.enter_context(tc.tile_pool(name="rpool", bufs=4))
        outp = ctx.enter_context(tc.tile_pool(name="outp", bufs=2))
        ptcs = ctx.enter_context(tc.tile_pool(name="ptcs", bufs=2))

        def load_bias(src, cols, tag):
            t = small.tile([P, cols], F32, tag=tag, name=tag)
            nc.gpsimd.dma_start(out=t, in_=src)
            return t

        b1_sb = load_bias(b1, KD, "b1")
        b2_sb = load_bias(b2, KB, "b2")
        b3_sb = load_bias(b3, KD, "b3")
        b4_sb = load_bias(b4, KD, "b4")
        bq_sb = load_bias(bq, KD, "bq")
        bk_sb = load_bias(bk, KD, "bk")

        # bv replicated across partitions for the final (exact, fp32) add
        bv_rep = small.tile([P, D], F32, tag="bvrep", name="bvrep")
        bv_bcast = bass.AP(tensor=bv.tensor, offset=bv.offset,
                           ap=[[0, P]] + list(bv.ap))
        nc.gpsimd.dma_start(out=bv_rep, in_=bv_bcast)

        # fp8 pair-packed ones column for the F-rowsum matmul
        ones8 = small.tile([P, 2, 1], F8, tag="ones8", name="ones8")
        nc.vector.memset(ones8, 1.0)
        # ln(16) per-partition bias column for the shifted exp
        lna = small.tile([P, 1], F32, tag="lna", name="lna")
        nc.vector.memset(lna, LN_ALPHA)
        # bf16 all-ones block used to replicate ysum along the free dim
        ones_bf = small.tile([P, P], BF, tag="onesbf", name="onesbf")
        nc.vector.memset(ones_bf, 1.0)

        def alloc_pairs(pool, pairs, n, tag, dt=F8):
            """fp8 pair-packed tiles [P, 2, n]."""
            return [pool.tile([P, 2, n], dt, tag=f"{tag}{t}", name=f"{tag}{t}")
                    for t in range(pairs)]

        def load_pairs(tiles, src, n, eng=None):
            eng = eng or nc.sync
            for t, tl in enumerate(tiles):
                eng.dma_start(
                    out=tl,
                    in_=src[t * P:(t + 1) * P, :].rearrange(
                        "p (r s) -> p r s", r=2))

        def act_block(dst, ps, func, bias_ap, on_vector):
            """relu/identity(ps + bias) -> fp8 dst, on Scalar or Vector."""
            if not on_vector:
                nc.scalar.activation(dst, ps, func, bias=bias_ap, scale=1.0)
            elif func == AF.Relu:
                nc.vector.tensor_scalar(dst, ps, bias_ap, 0.0,
                                        op0=ALU.add, op1=ALU.max)
            else:
                nc.vector.tensor_scalar_add(dst, ps, bias_ap)

        def fm_layer8(psum, src8, w8, pairs, mtiles, bias_sb, func, dst8):
            """fp8 DoubleRow feature-major layer into pair-packed fp8 dst."""
            for m in range(mtiles):
                pss = [psum.tile([P, NB], F32, tag="mm", name="mm")
                       for _ in range(NBLK)]
                for t in range(pairs):
                    lhs = w8[t][:, :, m * P:(m + 1) * P]
                    for tb in range(NBLK):
                        nc.tensor.matmul(pss[tb], lhs,
                                         src8[t][:, :, tb * NB:(tb + 1) * NB],
                                         start=(t == 0), stop=(t == pairs - 1),
                                         perf_mode=DR)
                for tb in range(NBLK):
                    dst = dst8[m // 2][:, m % 2, tb * NB:(tb + 1) * NB]
                    act_block(dst, pss[tb], func, bias_sb[:, m:m + 1],
                              on_vector=(tb % 2 == 1))

        # ------ persistent attention operands (q8, k8, v8) + ysum ------
        with tc.tile_pool(name="pq", bufs=1) as pq, \
             tc.tile_pool(name="pk", bufs=1) as pk, \
             tc.tile_pool(name="pv8", bufs=1) as pv8, \
             tc.tile_pool(name="pys", bufs=1) as pys:
            q8 = alloc_pairs(pq, PD, S, "q8")
            k8 = alloc_pairs(pk, PD, S, "k8")
            v8 = alloc_pairs(pv8, PT, D, "v8")
            ys32 = [pys.tile([P, 1], F32, tag=f"ys{t}", name=f"ys{t}")
                    for t in range(KD)]
            ysrep = [pys.tile([P, P], BF, tag=f"yr{t}", name=f"yr{t}")
                     for t in range(KD)]

            # y-side operands span stages A+B, freed before stage C
            with tc.tile_pool(name="py", bufs=1) as py, \
                 tc.tile_pool(name="pwk", bufs=1) as pwk, \
                 tc.tile_pool(name="pwv8", bufs=1) as pwv8:
                y8 = alloc_pairs(py, PD, S, "y8")
                wk8 = alloc_pairs(pwk, PD, D, "wk8")
                wv8 = alloc_pairs(pwv8, PD, D, "wv8")

                # ---------------- Stage A: x-MLP -> q8 (in SBUF) -------------
                with tc.tile_pool(name="wx", bufs=1) as wx, \
                     tc.tile_pool(name="px", bufs=1) as px, \
                     tc.tile_pool(name="phA", bufs=1) as phA, \
                     tc.tile_pool(name="phB", bufs=1) as phB, \
                     tc.tile_pool(name="psA", bufs=8, space="PSUM") as psA:
                    x8 = alloc_pairs(px, PD, S, "x8")
                    w18 = alloc_pairs(wx, PD, D, "w18")
                    # first-needed tiles first: interleave x8 / W1 pair loads
                    for t in range(PD):
                        nc.sync.dma_start(
                            out=x8[t], in_=x8d[t * P:(t + 1) * P, :].rearrange(
                                "p (r s) -> p r s", r=2))
                        nc.sync.dma_start(
                            out=w18[t], in_=W1d[t * P:(t + 1) * P, :].rearrange(
                                "p (r s) -> p r s", r=2))
                    w28 = alloc_pairs(wx, PD, DB, "w28")
                    load_pairs(w28, W2d, DB)
                    w38 = alloc_pairs(wx, PB, D, "w38")
                    load_pairs(w38, W3d, D)
                    w48 = alloc_pairs(wx, PD, D, "w48")
                    load_pairs(w48, W4d, D)
                    wq8 = alloc_pairs(wx, PD, D, "wq8")
                    load_pairs(wq8, Wqd, D)
                    # y-side prefetch (queued behind stage A's needs)
                    load_pairs(y8, y8d, S)
                    load_pairs(wk8, Wkd, D)
                    load_pairs(wv8, Wv8d, D)

                    h18 = alloc_pairs(phA, PD, S, "ha")
                    h28 = alloc_pairs(phB, PB, S, "hb")
                    h38 = alloc_pairs(phA, PD, S, "ha")   # reuse phA slots
                    h48 = alloc_pairs(phB, PD, S, "hb")   # grow phB
                    fm_layer8(psA, x8, w18, PD, KD, b1_sb, AF.Relu, h18)
                    fm_layer8(psA, h18, w28, PD, KB, b2_sb, AF.Relu, h28)
                    fm_layer8(psA, h28, w38, PB, KD, b3_sb, AF.Relu, h38)
                    fm_layer8(psA, h38, w48, PD, KD, b4_sb, AF.Relu, h48)
                    fm_layer8(psA, h48, wq8, PD, KD, bq_sb, AF.Identity, q8)

                # -------- Stage B: y -> k8 (fp8), v8 (fp8), ysum -------------
                with tc.tile_pool(name="pyT", bufs=2) as pyT, \
                     tc.tile_pool(name="psBk", bufs=4, space="PSUM") as psBk, \
                     tc.tile_pool(name="psBv", bufs=2, space="PSUM") as psBv:
                    # stream yT through a small pool, reduce to ysum tiles,
                    # replicate along free dim for the colsum matmul
                    for t in range(KD):
                        yt = pyT.tile([P, S], BF, tag="yt", name="yt")
                        nc.sync.dma_start(out=yt,
                                          in_=yTd[t * P:(t + 1) * P, :])
                        nc.vector.tensor_reduce(ys32[t], yt,
                                                axis=mybir.AxisListType.X,
                                                op=ALU.add)
                        nc.vector.tensor_scalar_mul(ysrep[t], ones_bf,
                                                    ys32[t])
                    # k^T in fp8 pairs (feature-major, bias per-partition)
                    for m in range(KD):
                        pss = [psBk.tile([P, NB], F32, tag="kk", name="kk")
                               for _ in range(NBLK)]
                        for t in range(PD):
                            lhs = wk8[t][:, :, m * P:(m + 1) * P]
                            for tb in range(NBLK):
                                nc.tensor.matmul(
                                    pss[tb], lhs,
                                    y8[t][:, :, tb * NB:(tb + 1) * NB],
                                    start=(t == 0), stop=(t == PD - 1),
                                    perf_mode=DR)
                        for tb in range(NBLK):
                            act_block(k8[m // 2][:, m % 2,
                                                 tb * NB:(tb + 1) * NB],
                                      pss[tb], AF.Identity, bk_sb[:, m:m + 1],
                                      on_vector=(tb % 2 == 1))
                    # v (token-major fp8 DR, bias-free: bv folded into the
                    # output stage), stored kv-pair-packed for attn@v
                    for tq in range(NT):
                        pv_ = psBv.tile([P, D], F32, tag="vv", name="vv")
                        for t in range(PD):
                            lhs = y8[t][:, :, tq * P:(tq + 1) * P]
                            for nb2 in range(2):
                                nc.tensor.matmul(
                                    pv_[:, nb2 * NB:(nb2 + 1) * NB], lhs,
                                    wv8[t][:, :, nb2 * NB:(nb2 + 1) * NB],
                                    start=(t == 0), stop=(t == PD - 1),
                                    perf_mode=DR)
                        nc.scalar.activation(v8[tq // 2][:, tq % 2, :], pv_,
                                             AF.Identity, bias=0.0, scale=1.0)

            # ---------------- Stage C: attention ----------------
            with tc.tile_pool(name="pwvb", bufs=1) as pwvb, \
                 tc.tile_pool(name="pE", bufs=2) as pE, \
                 tc.tile_pool(name="pt32", bufs=4) as pt32, \
                 tc.tile_pool(name="pcs", bufs=1) as pcs, \
                 tc.tile_pool(name="psCs", bufs=3, space="PSUM") as psCs, \
                 tc.tile_pool(name="psCo", bufs=2, space="PSUM") as psCo, \
                 tc.tile_pool(name="psCS", bufs=1, space="PSUM") as psCS:
                # Wv bf16 (for the high-precision colsum) loads under the
                # half-0 scores compute; the colsum matmuls run after them
                wvb = [pwvb.tile([P, D], BF, tag=f"wvb{t}", name=f"wvb{t}")
                       for t in range(KD)]
                for t in range(KD):
                    nc.gpsimd.dma_start(out=wvb[t],
                                        in_=Wvbd[t * P:(t + 1) * P, :])
                cs16 = pcs.tile([P, D], F32, tag="cs16", name="cs16")

                for half in range(2):
                    qoff = half * HALF
                    # F'^T = exp(scale*k@q^T + ln16) - 16 in fp8 pairs
                    et8 = alloc_pairs(pE, PT, HALF, "e")
                    for tk in range(NT):
                        for qb in range(HALF // NB):
                            ps = psCs.tile([P, NB], F32, tag="sc", name="sc")
                            for t in range(PD):
                                nc.tensor.matmul(
                                    ps, k8[t][:, :, tk * P:(tk + 1) * P],
                                    q8[t][:, :,
                                          qoff + qb * NB:qoff + (qb + 1) * NB],
                                    start=(t == 0), stop=(t == PD - 1),
                                    perf_mode=DR)
                            t32 = pt32.tile([P, NB], F32, tag="t32",
                                            name="t32")
                            nc.scalar.activation(t32, ps, AF.Exp,
                                                 bias=lna, scale=SCALE)
                            nc.vector.tensor_scalar_sub(
                                et8[tk // 2][:, tk % 2,
                                             qb * NB:(qb + 1) * NB],
                                t32, ALPHA)
                    if half == 0:
                        # CS16[p, d] = 16 * sum_k v0[k, d], identical on
                        # every partition: stationary = replicated ysum
                        for blk in range(2):
                            psc = psCs.tile([P, NB], F32, tag="sc",
                                            name="cs")
                            for t in range(KD):
                                nc.tensor.matmul(
                                    psc, ysrep[t],
                                    wvb[t][:, blk * NB:(blk + 1) * NB],
                                    start=(t == 0), stop=(t == KD - 1))
                            nc.scalar.mul(cs16[:, blk * NB:(blk + 1) * NB],
                                          psc, ALPHA)
                    # out rows: (F'@v + CS16) / (32768 + rowsum F') + bv
                    for tq8 in range(HALF // P):
                        tq = half * (HALF // P) + tq8
                        po = psCo.tile([P, D], F32, tag="oo", name="oo")
                        pS = psCS.tile([P, 1], F32, tag="ss", name="ss")
                        for t in range(PT):
                            lhs = et8[t][:, :, tq8 * P:(tq8 + 1) * P]
                            nc.tensor.matmul(po[:, 0:NB], lhs,
                                             v8[t][:, :, 0:NB],
                                             start=(t == 0),
                                             stop=(t == PT - 1),
                                             perf_mode=DR)
                            nc.tensor.matmul(po[:, NB:D], lhs,
                                             v8[t][:, :, NB:D],
                                             start=(t == 0),
                                             stop=(t == PT - 1),
                                             perf_mode=DR)
                            nc.tensor.matmul(pS, lhs, ones8,
                                             start=(t == 0),
                                             stop=(t == PT - 1),
                                             perf_mode=DR)
                        den = rpool.tile([P, 1], F32, tag="dn", name="dn")
                        nc.vector.tensor_scalar_add(den, pS, DEN_CONST)
                        rinv = rpool.tile([P, 1], F32, tag="ri", name="ri")
                        nc.vector.reciprocal(rinv, den)
                        # normalize + store in half-D chunks so the first
                        # DMA overlaps the second chunk's vector work;
                        # bf16 store halves the output DMA traffic
                        ot = outp.tile([P, D], BF, tag="ot", name="ot")
                        for ob in range(2):
                            sl = slice(ob * NB, (ob + 1) * NB)
                            tcs = ptcs.tile([P, NB], F32, tag="tcs",
                                            name="tcs")
                            nc.vector.scalar_tensor_tensor(
                                tcs, cs16[:, sl], rinv, bv_rep[:, sl],
                                op0=ALU.mult, op1=ALU.add)
                            nc.vector.scalar_tensor_tensor(
                                ot[:, sl], po[:, sl], rinv, tcs,
                                op0=ALU.mult, op1=ALU.add)
                            nc.sync.dma_start(
                                out=out[tq * P:(tq + 1) * P, sl],
                                in_=ot[:, sl])

    nc.compile()
    _NC = nc
    return nc


def _pack8(w):
    """[K, N] -> DoubleRow pair-packed fp8 [K/2, 2N]:
    out[t*128+p, r*N+m] = w[(2t+r)*128+p, m]."""
    K, N = w.shape
    return np.ascontiguousarray(
        w.astype(FP8).reshape(K // 256, 2, 128, N)
        .transpose(0, 2, 1, 3).reshape(K // 2, 2 * N))


def make_in_maps(inputs):
    """Host-side prep: per-core batch shard, fp8/bf16 casts + pair packing,
    feature-major transposes of x/y, bias relayout."""
    x = np.asarray(inputs["x"])
    y = np.asarray(inputs["y"])
    shared = {}
    for k in ("W1", "W2", "W3", "W4", "Wq", "Wk"):
        shared[k] = _pack8(np.asarray(inputs[k]).astype(np.float32))
    wv = np.asarray(inputs["Wv"]).astype(np.float32)
    shared["Wv8"] = _pack8(wv)
    shared["Wvb"] = np.ascontiguousarray(wv.astype(BF16))
    for k, nt in (("b1", KD), ("b2", KB), ("b3", KD), ("b4", KD),
                  ("bq", KD), ("bk", KD)):
        shared[k] = np.ascontiguousarray(
            np.asarray(inputs[k]).astype(np.float32).reshape(nt, P).T)
    shared["bv"] = np.ascontiguousarray(
        np.asarray(inputs["bv"]).astype(np.float32).reshape(D))
    in_maps = []
    for b in range(x.shape[0]):
        m = dict(shared)
        xT = np.ascontiguousarray(x[b].T)
        yT = np.ascontiguousarray(y[b].T)
        m["x8"] = _pack8(xT)
        m["y8"] = _pack8(yT)
        m["yT"] = yT.astype(BF16)
        in_maps.append(m)
    return in_maps


def kernel(**inputs):
    from concourse.bass_utils import run_bass_kernel_spmd

    nc = build_nc()
    in_maps = make_in_maps(inputs)
    res = run_bass_kernel_spmd(nc, in_maps, list(range(len(in_maps))))
    return np.stack([np.asarray(r["out"], dtype=np.float32)
                     for r in res.results])


# revision 17
# speedup vs baseline: 1.0114x; 1.0114x over previous
"""Trainium2 Bass kernel for nn_CrossAttention (4-layer MLP -> cross-attention).

Sharding: data-parallel across batch B=8, one batch element per NeuronCore.

Layout strategy (per core):
  - activations flow feature-major (transposed): the MLP chain
    h_{l+1}^T = W_l^T @ h_l^T needs no transposes (W natural [K,M] = lhsT),
  - scores are computed transposed (scores^T = k @ q^T, kv on partitions),
  - v is computed token-major from y8/Wv8 (fp8 DR) and stored kv-pair-packed,
  - the attention output uses the decomposition
        E = exp(s) = 1 + F,   out = (colsum(v) + F @ v) / (2048 + sum F) + bv
    with F' = 16*F = exp(s + ln16) - 16 stored in fp8 (scale 16 keeps F'
    clear of the e4m3 subnormal floor); colsum(v) = (16*ysum^T Wv) in bf16
    from the true y (ysum streamed+reduced from yT), so the fp8 noise of v
    only enters multiplied by |F| ~ 0.02 instead of E ~ 1.  All the alpha
    factors cancel: out = (po + CS16)/(32768 + pS') + bv.

Precision: every big matmul runs fp8(e4m3) DoubleRow (2 contraction rows
per PE pass -> 2x FLOP rate at the same column stream rate); errors on the
MLP/q/k path only perturb attention weights, the F-decomposition suppresses
the v/E quantization error by sigma_F ~ 0.02. All accumulation fp32 in PSUM.

fp8 operands are pair-packed for DoubleRow: logical contraction index
k = (2t+r)*128+p lives in tile t, partition p, middle index r, i.e. SBUF
tiles [128, 2, N] (weights pre-packed on host to [K/2, 2*N]).

Activations are split between the Scalar and Vector engines (alternating
free-dim blocks) so neither stalls the tensor engine.
"""

import sys

if "/opt/trn_rl_repo" not in sys.path:
    sys.path.insert(0, "/opt/trn_rl_repo")

import numpy as np
import ml_dtypes

P = 128
D = 1024
DB = 512
S = 2048
KD = D // P       # 8 feature tiles of 128
KB = DB // P      # 4
PD = KD // 2      # 4 fp8 pair-tiles for a 1024 contraction
PB = KB // 2      # 2 for 512
NT = S // P       # 16 token tiles
PT = NT // 2      # 8 kv pair tiles
NB = 512          # moving-operand free-dim block
NBLK = S // NB    # 4 token blocks
HALF = S // 2     # q processed in 2 halves during attention
NCORES = 8
SCALE = float(1.0 / np.sqrt(D))
ALPHA = 16.0
LN_ALPHA = float(np.log(ALPHA))
DEN_CONST = ALPHA * S   # 32768

BF16 = ml_dtypes.bfloat16
FP8 = ml_dtypes.float8_e4m3

_NC = None


def build_nc():
    """Build + compile the per-core Bass program (cached)."""
    global _NC
    if _NC is not None:
        return _NC

    from contextlib import ExitStack
    import concourse.bass as bass
    import concourse.tile as tile
    from concourse import bacc, mybir

    BF = mybir.dt.bfloat16
    F8 = mybir.dt.float8e4
    F32 = mybir.dt.float32
    AF = mybir.ActivationFunctionType
    ALU = mybir.AluOpType
    DR = mybir.MatmulPerfMode.DoubleRow

    nc = bacc.Bacc("TRN2", target_bir_lowering=False, debug=False,
                   num_devices=NCORES)

    def din(name, shape, dt):
        return nc.dram_tensor(name, shape, dt, kind="ExternalInput").ap()

    # fp8 operands arrive pair-packed: [K/2, 2*N]
    x8d = din("x8", [D // 2, 2 * S], F8)
    y8d = din("y8", [D // 2, 2 * S], F8)
    yTd = din("yT", [D, S], BF)
    W1d = din("W1", [D // 2, 2 * D], F8)
    W2d = din("W2", [D // 2, 2 * DB], F8)
    W3d = din("W3", [DB // 2, 2 * D], F8)
    W4d = din("W4", [D // 2, 2 * D], F8)
    Wqd = din("Wq", [D // 2, 2 * D], F8)
    Wkd = din("Wk", [D // 2, 2 * D], F8)
    Wv8d = din("Wv8", [D // 2, 2 * D], F8)
    Wvbd = din("Wvb", [D, D], BF)
    b1 = din("b1", [P, KD], F32)
    b2 = din("b2", [P, KB], F32)
    b3 = din("b3", [P, KD], F32)
    b4 = din("b4", [P, KD], F32)
    bq = din("bq", [P, KD], F32)
    bk = din("bk", [P, KD], F32)
    bv = din("bv", [D], F32)
    out = nc.dram_tensor("out", [S, D], F32, kind="ExternalOutput").ap()

    with tile.TileContext(nc) as tc, ExitStack() as ctx:
        small = ctx.enter_context(tc.tile_pool(name="small", bufs=1))
        rpool = ctx.enter_context(tc.tile_pool(name="rpool", bufs=4))
        outp = ctx.enter_context(tc.tile_pool(name="outp", bufs=2))
        ptcs = ctx.enter_context(tc.tile_pool(name="ptcs", bufs=2))

        def load_bias(src, cols, tag):
            t = small.tile([P, cols], F32, tag=tag, name=tag)
            nc.gpsimd.dma_start(out=t, in_=src)
            return t

        b1_sb = load_bias(b1, KD, "b1")
        b2_sb = load_bias(b2, KB, "b2")
        b3_sb = load_bias(b3, KD, "b3")
        b4_sb = load_bias(b4, KD, "b4")
        bq_sb = load_bias(bq, KD, "bq")
        bk_sb = load_bias(bk, KD, "bk")

        # bv replicated across partitions for the final (exact, fp32) add
        bv_rep = small.tile([P, D], F32, tag="bvrep", name="bvrep")
        bv_bcast = bass.AP(tensor=bv.tensor, offset=bv.offset,
                           ap=[[0, P]] + list(bv.ap))
        nc.gpsimd.dma_start(out=bv_rep, in_=bv_bcast)

        # fp8 pair-packed ones column for the F-rowsum matmul
        ones8 = small.tile([P, 2, 1], F8, tag="ones8", name="ones8")
        nc.vector.memset(ones8, 1.0)
        # ln(16) per-partition bias column for the shifted exp
        lna = small.tile([P, 1], F32, tag="lna", name="lna")
        nc.vector.memset(lna, LN_ALPHA)
        # bf16 all-ones block used to replicate ysum along the free dim
        ones_bf = small.tile([P, P], BF, tag="onesbf", name="onesbf")
        nc.vector.memset(ones_bf, 1.0)

        def alloc_pairs(pool, pairs, n, tag, dt=F8):
            """fp8 pair-packed tiles [P, 2, n]."""
            return [pool.tile([P, 2, n], dt, tag=f"{tag}{t}", name=f"{tag}{t}")
                    for t in range(pairs)]

        def load_pairs(tiles, src, n, eng=None):
            eng = eng or nc.sync
            for t, tl in enumerate(tiles):
                eng.dma_start(
                    out=tl,
                    in_=src[t * P:(t + 1) * P, :].rearrange(
                        "p (r s) -> p r s", r=2))

        def act_block(dst, ps, func, bias_ap, on_vector):
            """relu/identity(ps + bias) -> fp8 dst, on Scalar or Vector."""
            if not on_vector:
                nc.scalar.activation(dst, ps, func, bias=bias_ap, scale=1.0)
            elif func == AF.Relu:
                nc.vector.tensor_scalar(dst, ps, bias_ap, 0.0,
                                        op0=ALU.add, op1=ALU.max)
            else:
                nc.vector.tensor_scalar_add(dst, ps, bias_ap)

        def fm_layer8(psum, src8, w8, pairs, mtiles, bias_sb, func, dst8):
            """fp8 DoubleRow feature-major layer into pair-packed fp8 dst."""
            for m in range(mtiles):
                pss = [psum.tile([P, NB], F32, tag="mm", name="mm")
                       for _ in range(NBLK)]
                for t in range(pairs):
                    lhs = w8[t][:, :, m * P:(m + 1) * P]
                    for tb in range(NBLK):
                        nc.tensor.matmul(pss[tb], lhs,
                                         src8[t][:, :, tb * NB:(tb + 1) * NB],
                                         start=(t == 0), stop=(t == pairs - 1),
                                         perf_mode=DR)
                for tb in range(NBLK):
                    dst = dst8[m // 2][:, m % 2, tb * NB:(tb + 1) * NB]
                    act_block(dst, pss[tb], func, bias_sb[:, m:m + 1],
                              on_vector=(tb % 2 == 1))

        # ------ persistent attention operands (q8, k8, v8) + ysum ------
        with tc.tile_pool(name="pq", bufs=1) as pq, \
             tc.tile_pool(name="pk", bufs=1) as pk, \
             tc.tile_pool(name="pv8", bufs=1) as pv8, \
             tc.tile_pool(name="pys", bufs=1) as pys:
            q8 = alloc_pairs(pq, PD, S, "q8")
            k8 = alloc_pairs(pk, PD, S, "k8")
            v8 = alloc_pairs(pv8, PT, D, "v8")
            ys32 = [pys.tile([P, 1], F32, tag=f"ys{t}", name=f"ys{t}")
                    for t in range(KD)]
            ysrep = [pys.tile([P, P], BF, tag=f"yr{t}", name=f"yr{t}")
                     for t in range(KD)]

            # y-side operands span stages A+B, freed before stage C
            with tc.tile_pool(name="py", bufs=1) as py, \
                 tc.tile_pool(name="pwk", bufs=1) as pwk, \
                 tc.tile_pool(name="pwv8", bufs=1) as pwv8:
                y8 = alloc_pairs(py, PD, S, "y8")
                wk8 = alloc_pairs(pwk, PD, D, "wk8")
                wv8 = alloc_pairs(pwv8, PD, D, "wv8")

                # ---------------- Stage A: x-MLP -> q8 (in SBUF) -------------
                with tc.tile_pool(name="wx", bufs=1) as wx, \
                     tc.tile_pool(name="px", bufs=1) as px, \
                     tc.tile_pool(name="phA", bufs=1) as phA, \
                     tc.tile_pool(name="phB", bufs=1) as phB, \
                     tc.tile_pool(name="psA", bufs=8, space="PSUM") as psA:
                    x8 = alloc_pairs(px, PD, S, "x8")
                    w18 = alloc_pairs(wx, PD, D, "w18")
                    # first-needed tiles first: interleave x8 / W1 pair loads
                    for t in range(PD):
                        nc.sync.dma_start(
                            out=x8[t], in_=x8d[t * P:(t + 1) * P, :].rearrange(
                                "p (r s) -> p r s", r=2))
                        nc.sync.dma_start(
                            out=w18[t], in_=W1d[t * P:(t + 1) * P, :].rearrange(
                                "p (r s) -> p r s", r=2))
                    w28 = alloc_pairs(wx, PD, DB, "w28")
                    load_pairs(w28, W2d, DB)
                    w38 = alloc_pairs(wx, PB, D, "w38")
                    load_pairs(w38, W3d, D)
                    w48 = alloc_pairs(wx, PD, D, "w48")
                    load_pairs(w48, W4d, D)
                    wq8 = alloc_pairs(wx, PD, D, "wq8")
                    load_pairs(wq8, Wqd, D)
                    # y-side prefetch (queued behind stage A's needs)
                    load_pairs(y8, y8d, S)
                    load_pairs(wk8, Wkd, D)
                    load_pairs(wv8, Wv8d, D)

                    h18 = alloc_pairs(phA, PD, S, "ha")
                    h28 = alloc_pairs(phB, PB, S, "hb")
                    h38 = alloc_pairs(phA, PD, S, "ha")   # reuse phA slots
                    h48 = alloc_pairs(phB, PD, S, "hb")   # grow phB
                    fm_layer8(psA, x8, w18, PD, KD, b1_sb, AF.Relu, h18)
                    fm_layer8(psA, h18, w28, PD, KB, b2_sb, AF.Relu, h28)
                    fm_layer8(psA, h28, w38, PB, KD, b3_sb, AF.Relu, h38)
                    fm_layer8(psA, h38, w48, PD, KD, b4_sb, AF.Relu, h48)
                    fm_layer8(psA, h48, wq8, PD, KD, bq_sb, AF.Identity, q8)

                # -------- Stage B: y -> k8 (fp8), v8 (fp8), ysum -------------
                with tc.tile_pool(name="pyT", bufs=2) as pyT, \
                     tc.tile_pool(name="psBk", bufs=4, space="PSUM") as psBk, \
                     tc.tile_pool(name="psBv", bufs=2, space="PSUM") as psBv:
                    # stream yT through a small pool, reduce to ysum tiles,
                    # replicate along free dim for the colsum matmul
                    for t in range(KD):
                        yt = pyT.tile([P, S], BF, tag="yt", name="yt")
                        nc.sync.dma_start(out=yt,
                                          in_=yTd[t * P:(t + 1) * P, :])
                        nc.vector.tensor_reduce(ys32[t], yt,
                                                axis=mybir.AxisListType.X,
                                                op=ALU.add)
                        nc.vector.tensor_scalar_mul(ysrep[t], ones_bf,
                                                    ys32[t])
                    # k^T in fp8 pairs (feature-major, bias per-partition)
                    for m in range(KD):
                        pss = [psBk.tile([P, NB], F32, tag="kk", name="kk")
                               for _ in range(NBLK)]
                        for t in range(PD):
                            lhs = wk8[t][:, :, m * P:(m + 1) * P]
                            for tb in range(NBLK):
                                nc.tensor.matmul(
                                    pss[tb], lhs,
                                    y8[t][:, :, tb * NB:(tb + 1) * NB],
                                    start=(t == 0), stop=(t == PD - 1),
                                    perf_mode=DR)
                        for tb in range(NBLK):
                            act_block(k8[m // 2][:, m % 2,
                                                 tb * NB:(tb + 1) * NB],
                                      pss[tb], AF.Identity, bk_sb[:, m:m + 1],
                                      on_vector=(tb % 2 == 1))
                    # v (token-major fp8 DR, bias-free: bv folded into the
                    # output stage), stored kv-pair-packed for attn@v
                    for tq in range(NT):
                        pv_ = psBv.tile([P, D], F32, tag="vv", name="vv")
                        for t in range(PD):
                            lhs = y8[t][:, :, tq * P:(tq + 1) * P]
                            for nb2 in range(2):
                                nc.tensor.matmul(
                                    pv_[:, nb2 * NB:(nb2 + 1) * NB], lhs,
                                    wv8[t][:, :, nb2 * NB:(nb2 + 1) * NB],
                                    start=(t == 0), stop=(t == PD - 1),
                                    perf_mode=DR)
                        nc.scalar.activation(v8[tq // 2][:, tq % 2, :], pv_,
                                             AF.Identity, bias=0.0, scale=1.0)

            # ---------------- Stage C: attention ----------------
            with tc.tile_pool(name="pwvb", bufs=1) as pwvb, \
                 tc.tile_pool(name="pE", bufs=2) as pE, \
                 tc.tile_pool(name="pt32", bufs=4) as pt32, \
                 tc.tile_pool(name="pcs", bufs=1) as pcs, \
                 tc.tile_pool(name="psCs", bufs=3, space="PSUM") as psCs, \
                 tc.tile_pool(name="psCo", bufs=2, space="PSUM") as psCo, \
                 tc.tile_pool(name="psCS", bufs=1, space="PSUM") as psCS:
                # Wv bf16 (for the high-precision colsum) loads under the
                # half-0 scores compute; the colsum matmuls run after them
                wvb = [pwvb.tile([P, D], BF, tag=f"wvb{t}", name=f"wvb{t}")
                       for t in range(KD)]
                for t in range(KD):
                    nc.gpsimd.dma_start(out=wvb[t],
                                        in_=Wvbd[t * P:(t + 1) * P, :])
                cs16 = pcs.tile([P, D], F32, tag="cs16", name="cs16")

                for half in range(2):
                    qoff = half * HALF
                    # F'^T = exp(scale*k@q^T + ln16) - 16 in fp8 pairs
                    et8 = alloc_pairs(pE, PT, HALF, "e")
                    for tk in range(NT):
                        for qb in range(HALF // NB):
                            ps = psCs.tile([P, NB], F32, tag="sc", name="sc")
                            for t in range(PD):
                                nc.tensor.matmul(
                                    ps, k8[t][:, :, tk * P:(tk + 1) * P],
                                    q8[t][:, :,
                                          qoff + qb * NB:qoff + (qb + 1) * NB],
                                    start=(t == 0), stop=(t == PD - 1),
                                    perf_mode=DR)
                            t32 = pt32.tile([P, NB], F32, tag="t32",
                                            name="t32")
                            nc.scalar.activation(t32, ps, AF.Exp,
                                                 bias=lna, scale=SCALE)
                            nc.vector.tensor_scalar_sub(
                                et8[tk // 2][:, tk % 2,
                                             qb * NB:(qb + 1) * NB],
                                t32, ALPHA)
                    if half == 0:
                        # CS16[p, d] = 16 * sum_k v0[k, d], identical on
                        # every partition: stationary = replicated ysum
                        for blk in range(2):
                            psc = psCs.tile([P, NB], F32, tag="sc",
                                            name="cs")
                            for t in range(KD):
                                nc.tensor.matmul(
                                    psc, ysrep[t],
                                    wvb[t][:, blk * NB:(blk + 1) * NB],
                                    start=(t == 0), stop=(t == KD - 1))
                            nc.scalar.mul(cs16[:, blk * NB:(blk + 1) * NB],
                                          psc, ALPHA)
                    # out rows: (F'@v + CS16) / (32768 + rowsum F') + bv
                    for tq8 in range(HALF // P):
                        tq = half * (HALF // P) + tq8
                        po = psCo.tile([P, D], F32, tag="oo", name="oo")
                        pS = psCS.tile([P, 1], F32, tag="ss", name="ss")
                        for t in range(PT):
                            lhs = et8[t][:, :, tq8 * P:(tq8 + 1) * P]
                            nc.tensor.matmul(po[:, 0:NB], lhs,
                                             v8[t][:, :, 0:NB],
                                             start=(t == 0),
                                             stop=(t == PT - 1),
                                             perf_mode=DR)
                            nc.tensor.matmul(po[:, NB:D], lhs,
                                             v8[t][:, :, NB:D],
                                             start=(t == 0),
                                             stop=(t == PT - 1),
                                             perf_mode=DR)
                            nc.tensor.matmul(pS, lhs, ones8,
                                             start=(t == 0),
                                             stop=(t == PT - 1),
                                             perf_mode=DR)
                        den = rpool.tile([P, 1], F32, tag="dn", name="dn")
                        nc.vector.tensor_scalar_add(den, pS, DEN_CONST)
                        rinv = rpool.tile([P, 1], F32, tag="ri", name="ri")
                        nc.vector.reciprocal(rinv, den)
                        # normalize + store in half-D chunks so the first
                        # DMA overlaps the second chunk's vector work
                        ot = outp.tile([P, D], F32, tag="ot", name="ot")
                        for ob in range(2):
                            sl = slice(ob * NB, (ob + 1) * NB)
                            tcs = ptcs.tile([P, NB], F32, tag="tcs",
                                            name="tcs")
                            nc.vector.scalar_tensor_tensor(
                                tcs, cs16[:, sl], rinv, bv_rep[:, sl],
                                op0=ALU.mult, op1=ALU.add)
                            nc.vector.scalar_tensor_tensor(
                                ot[:, sl], po[:, sl], rinv, tcs,
                                op0=ALU.mult, op1=ALU.add)
                            nc.sync.dma_start(
                                out=out[tq * P:(tq + 1) * P, sl],
                                in_=ot[:, sl])

    nc.compile()
    _NC = nc
    return nc


def _pack8(w):
    """[K, N] -> DoubleRow pair-packed fp8 [K/2, 2N]:
    out[t*128+p, r*N+m] = w[(2t+r)*128+p, m]."""
    K, N = w.shape
    return np.ascontiguousarray(
        w.astype(FP8).reshape(K // 256, 2, 128, N)
        .transpose(0, 2, 1, 3).reshape(K // 2, 2 * N))


def make_in_maps(inputs):
    """Host-side prep: per-core batch shard, fp8/bf16 casts + pair packing,
    feature-major transposes of x/y, bias relayout."""
    x = np.asarray(inputs["x"])
    y = np.asarray(inputs["y"])
    shared = {}
    for k in ("W1", "W2", "W3", "W4", "Wq", "Wk"):
        shared[k] = _pack8(np.asarray(inputs[k]).astype(np.float32))
    wv = np.asarray(inputs["Wv"]).astype(np.float32)
    shared["Wv8"] = _pack8(wv)
    shared["Wvb"] = np.ascontiguousarray(wv.astype(BF16))
    for k, nt in (("b1", KD), ("b2", KB), ("b3", KD), ("b4", KD),
                  ("bq", KD), ("bk", KD)):
        shared[k] = np.ascontiguousarray(
            np.asarray(inputs[k]).astype(np.float32).reshape(nt, P).T)
    shared["bv"] = np.ascontiguousarray(
        np.asarray(inputs["bv"]).astype(np.float32).reshape(D))
    in_maps = []
    for b in range(x.shape[0]):
        m = dict(shared)
        xT = np.ascontiguousarray(x[b].T)
        yT = np.ascontiguousarray(y[b].T)
        m["x8"] = _pack8(xT)
        m["y8"] = _pack8(yT)
        m["yT"] = yT.astype(BF16)
        in_maps.append(m)
    return in_maps


def kernel(**inputs):
    from concourse.bass_utils import run_bass_kernel_spmd

    nc = build_nc()
    in_maps = make_in_maps(inputs)
    res = run_bass_kernel_spmd(nc, in_maps, list(range(len(in_maps))))
    return np.stack([np.asarray(r["out"], dtype=np.float32)
                     for r in res.results])
